# revision 1
# baseline (speedup 1.0000x reference)
"""Cross-attention transformer block on 8 Trainium2 NeuronCores.

Sharding: pure data-parallel — batch B=8, one batch element per core, no
collectives. Weights are replicated (broadcast via per-core input maps).

Per-core dataflow is kept entirely "feature-major" ([D, L] layouts, feature
dim on partitions) so that:
  - layernorm gain/bias and QKV biases are per-partition scalars,
  - the attention mask is a per-partition bias on the exp() activation
    (scores are computed transposed: S^T[k, q]),
  - softmax denominators come out of the PV matmul for free via a
    ones-augmented V (row 64 of each head's PV output = sum_k exp),
  - LayerNorm mean/var are computed with ones-matmul partition reductions
    and broadcast back with K=1 matmuls,
  - no on-device transposes are needed anywhere; the host transposes the
    final [D, L] output back to [L, D].

Matmuls run in bf16 (fp32 PSUM accumulation); softmax skips the max
subtraction (scores are bounded, exp of masked -1e4 underflows to exactly 0).
"""

import numpy as np
import ml_dtypes

import concourse.bass as bass
import concourse.mybir as mybir
import concourse.tile as tile
from concourse import bacc
from contextlib import ExitStack

BF = mybir.dt.bfloat16
F32 = mybir.dt.float32
AF = mybir.ActivationFunctionType
ALU = mybir.AluOpType

D = 1024      # model dim
H = 16        # heads
HD = 64       # head dim
L = 1024      # seq len (Lq == Lk)
T = 8         # 128-row tiles in D
MF = 32       # 128-row tiles in 4*D
QC = 512      # q-chunk (free-dim tile for moving operands)
NQC = L // QC
SCALE = HD ** -0.5
EPS = 1e-5
NCORES = 8

_BUILt_CACHE = {}


def _build_program():
    nc = bacc.Bacc("TRN2", target_bir_lowering=False, debug=False)

    qt_bf = nc.declare_dram_parameter("qt_bf", [NQC, 128, T, QC], BF, isOutput=False)
    kvt = nc.declare_dram_parameter("kvt", [T, 128, L], BF, isOutput=False)
    wq = nc.declare_dram_parameter("wq", [T, 128, T, 128], BF, isOutput=False)
    wk = nc.declare_dram_parameter("wk", [T, 128, T, 128], BF, isOutput=False)
    wv = nc.declare_dram_parameter("wv", [2, 128, T, 512], BF, isOutput=False)
    wo = nc.declare_dram_parameter("wo", [T, 128, T, 128], BF, isOutput=False)
    w1 = nc.declare_dram_parameter("w1", [MF, 128, T, 128], BF, isOutput=False)
    w2 = nc.declare_dram_parameter("w2", [T, 128, MF, 128], BF, isOutput=False)
    bq_h = nc.declare_dram_parameter("bq", [128, T], F32, isOutput=False)
    bk_h = nc.declare_dram_parameter("bk", [128, T], F32, isOutput=False)
    bv_h = nc.declare_dram_parameter("bv", [L], F32, isOutput=False)
    bo_h = nc.declare_dram_parameter("bo", [128, T], F32, isOutput=False)
    b1_h = nc.declare_dram_parameter("b1", [128, MF], F32, isOutput=False)
    b2_h = nc.declare_dram_parameter("b2", [128, T], F32, isOutput=False)
    g1_h = nc.declare_dram_parameter("g1", [128, T], F32, isOutput=False)
    be1_h = nc.declare_dram_parameter("be1", [128, T], F32, isOutput=False)
    g2_h = nc.declare_dram_parameter("g2", [128, T], F32, isOutput=False)
    be2_h = nc.declare_dram_parameter("be2", [128, T], F32, isOutput=False)
    mb_h = nc.declare_dram_parameter("mb", [128, T], F32, isOutput=False)
    out_h = nc.declare_dram_parameter("out", [T, NQC, 128, QC], F32, isOutput=True)

    with tile.TileContext(nc) as tc, ExitStack() as ctx:
        sing = ctx.enter_context(tc.tile_pool(name="sing", bufs=1))
        big = ctx.enter_context(tc.tile_pool(name="big", bufs=1))
        wkq = ctx.enter_context(tc.tile_pool(name="wkq", bufs=2))
        w8k = ctx.enter_context(tc.tile_pool(name="w8k", bufs=2))
        qts = ctx.enter_context(tc.tile_pool(name="qts", bufs=1))
        atp = ctx.enter_context(tc.tile_pool(name="atp", bufs=4))
        rowp = ctx.enter_context(tc.tile_pool(name="rowp", bufs=1))
        lnp = ctx.enter_context(tc.tile_pool(name="lnp", bufs=3))
        tmp = ctx.enter_context(tc.tile_pool(name="tmp", bufs=3))
        sqp = ctx.enter_context(tc.tile_pool(name="sqp", bufs=2))
        resp = ctx.enter_context(tc.tile_pool(name="resp", bufs=1))
        outp = ctx.enter_context(tc.tile_pool(name="outp", bufs=2))
        acc = ctx.enter_context(tc.tile_pool(name="acc", bufs=2, space="PSUM"))
        stp = ctx.enter_context(tc.tile_pool(name="stp", bufs=2, space="PSUM"))
        ps_pv = ctx.enter_context(tc.tile_pool(name="ps_pv", bufs=2, space="PSUM"))

        # ---- constants / small inputs ----
        cols = {}
        for nm, h in [("bq", bq_h), ("bk", bk_h), ("bo", bo_h), ("b2", b2_h),
                      ("g1", g1_h), ("be1", be1_h), ("g2", g2_h), ("be2", be2_h),
                      ("mb", mb_h)]:
            csb = sing.tile([128, T], F32, name=f"{nm}_sb")
            nc.sync.dma_start(out=csb, in_=h[:, :])
            cols[nm] = csb
        b1_sb = sing.tile([128, MF], F32)
        nc.sync.dma_start(out=b1_sb, in_=b1_h[:, :])
        bv_ap = bv_h[:]
        bv_bc = sing.tile([128, L], F32)
        nc.sync.dma_start(
            out=bv_bc,
            in_=bass.AP(tensor=bv_ap.tensor, offset=bv_ap.offset,
                        ap=[[0, 128]] + list(bv_ap.ap)),
        )
        warmt = sing.tile([128, 1], F32, name="warmt")

        def warm(ap):
            nc.vector.tensor_copy(out=warmt, in_=ap)

        for nm in cols:
            warm(cols[nm][:, 0:1])
        warm(b1_sb[:, 0:1])
        warm(bv_bc[:, 0:1])
        ones_f = sing.tile([128, 128], F32)
        nc.vector.memset(ones_f, 1.0)
        invd = sing.tile([128, 1], BF)
        nc.vector.memset(invd, 1.0 / D)
        eps_sb = sing.tile([128, 1], F32)
        nc.vector.memset(eps_sb, EPS)

        # kvT resident; shares the 32KB "big" slot with hT later
        kvt_sb = big.tile([128, T, L], BF, tag="big")
        nc.sync.dma_start(out=kvt_sb,
                          in_=kvt[:, :, :].rearrange("t p k -> p t k"))

        # ---- K projection: kpT[dout, k] = (kv @ Wk + bk)^T, feature-major ----
        kpt = sing.tile([128, T, L], BF)
        for mj in range(T // 2):
            wkm2 = wkq.tile([128, 2, T, 128], BF, tag="wkq", name="wkm2")
            nc.sync.dma_start(out=wkm2,
                              in_=wk[2 * mj:2 * mj + 2].rearrange("a p t q -> p a t q"))
          
            for sub in range(2):
              m = 2 * mj + sub
              wkm = wkm2[:, sub]
              for c in range(2):
                  pk = acc.tile([128, 512], F32, tag="acc", name="pk")
                  for t in range(T):
                      nc.tensor.matmul(pk, lhsT=wkm[:, t, :],
                                       rhs=kvt_sb[:, t, c * 512:(c + 1) * 512],
                                       start=(t == 0), stop=(t == T - 1))
                  nc.scalar.activation(
                      out=kpt[:, m, c * 512:(c + 1) * 512], in_=pk,
                      func=AF.Identity, bias=cols["bk"][:, m:m + 1], scale=1.0)

        vaug = sing.tile([128, T, H, HD + 1], BF)
        for t in range(T):
            nc.vector.memset(vaug[:, t, :, HD:HD + 1], 1.0)

        # ---- V projection (emitted after qc0 attention so its dense matmuls
        # overlap the latency-bound attention pipeline) ----
        for c in range(2):
            wvc = w8k.tile([128, T, 512], BF, tag="w8k", name="wvc")
            nc.scalar.dma_start(out=wvc, in_=wv[c])
            for kt in range(T):
                pv_ = acc.tile([128, 512], F32, tag="acc", name="pv_")
                for t in range(T):
                    nc.tensor.matmul(pv_, lhsT=kvt_sb[:, t, kt * 128:(kt + 1) * 128],
                                     rhs=wvc[:, t, :],
                                     start=(t == 0), stop=(t == T - 1))
                nc.vector.tensor_add(
                    out=vaug[:, kt, c * 8:(c + 1) * 8, 0:HD],
                    in0=pv_.rearrange("p (h d) -> p h d", d=HD),
                    in1=bv_bc[:, c * 512:(c + 1) * 512].rearrange(
                        "p (h d) -> p h d", d=HD))

        def ln_rows(accr, nm):
            # accr[:,0,:] = mean, accr[:,1,:] = E[x^2]  (SBUF, fp32)
            vrow = lnp.tile([1, QC], F32, tag="lnrow_sb", name=f"vrow{nm}")
            nc.vector.tensor_mul(out=vrow, in0=accr[:, 0, :], in1=accr[:, 0, :])
            nc.vector.tensor_sub(out=vrow, in0=accr[:, 1, :], in1=vrow)
            srow = lnp.tile([1, QC], F32, tag="lnrow_sb", name=f"srow{nm}")
            nc.scalar.activation(out=srow, in_=vrow, func=AF.Sqrt,
                                 bias=eps_sb[0:1, :], scale=1.0)
            nc.vector.reciprocal(out=srow, in_=srow)
            mbc_ = acc.tile([128, QC], F32, tag="acc", name=f"mbc{nm}")
            nc.tensor.matmul(mbc_, lhsT=ones_f[0:1, :], rhs=accr[:, 0, :],
                             start=True, stop=True)
            rbc_ = acc.tile([128, QC], F32, tag="acc", name=f"rbc{nm}")
            nc.tensor.matmul(rbc_, lhsT=ones_f[0:1, :], rhs=srow,
                             start=True, stop=True)
            return mbc_, rbc_

        # ---- per-chunk phase emitters; ordered so both attention
        # phases precede both FFN phases (PE priority interleave) ----
        def emit_qa(qc):
            # Q projection (feature-major qpT[dout, q])
            qt_sb = qts.tile([128, T, QC], BF, tag="qt", name="qt_sb", bufs=2)
            nc.sync.dma_start(out=qt_sb, in_=qt_bf[qc])
            warm(qt_sb[:, 0, 0:1])
            qpt = qts.tile([128, T, QC], BF, tag="qpt", name="qpt")
            wq2 = None
            for m in range(T):
                if m % 2 == 0:
                    wq2 = wkq.tile([128, 2, T, 128], BF, tag="wkq", name="wq2")
                    nc.scalar.dma_start(
                        out=wq2,
                        in_=wq[m:m + 2].rearrange("a p t q -> p a t q"))
                wqm = wq2[:, m % 2]
                pq = acc.tile([128, QC], F32, tag="acc", name="pq")
                for t in range(T):
                    nc.tensor.matmul(pq, lhsT=wqm[:, t, :], rhs=qt_sb[:, t, :],
                                     start=(t == 0), stop=(t == T - 1))
                nc.scalar.activation(out=qpt[:, m, :], in_=pq, func=AF.Identity,
                                 bias=cols["bq"][:, m:m + 1], scale=1.0)

            # attention; head pair (2*hp, 2*hp+1) lives in partition halves of
            # dout-tile hp of qpt/kpt → row-tiled concurrent K=64 matmuls
            attout = qts.tile([128, T, QC], BF, tag="attout", name="attout")
            for hp in range(T):
                pv_e = ps_pv.tile([HD + 1, QC], F32, tag="pv", name="pv_e")
                pv_o = ps_pv.tile([HD + 1, QC], F32, tag="pv", name="pv_o")
                for kt in range(T):
                    st2 = stp.tile([128, 2, QC], F32, tag="st", name="st2")
                    nc.tensor.matmul(st2[:, 0, :],
                                     lhsT=kpt[0:64, hp, kt * 128:(kt + 1) * 128],
                                     rhs=qpt[0:64, hp, :], start=True, stop=True)
                    nc.tensor.matmul(st2[:, 1, :],
                                     lhsT=kpt[64:128, hp, kt * 128:(kt + 1) * 128],
                                     rhs=qpt[64:128, hp, :], start=True, stop=True)
                    at2 = atp.tile([128, 2, QC], BF, tag="at", name="at2")
                    nc.scalar.activation(out=at2, in_=st2, func=AF.Exp,
                                         bias=cols["mb"][:, kt:kt + 1], scale=SCALE)
                    nc.tensor.matmul(pv_e, lhsT=vaug[:, kt, 2 * hp, :],
                                     rhs=at2[:, 0, :],
                                     start=(kt == 0), stop=(kt == T - 1))
                    nc.tensor.matmul(pv_o, lhsT=vaug[:, kt, 2 * hp + 1, :],
                                     rhs=at2[:, 1, :],
                                     start=(kt == 0), stop=(kt == T - 1))
                bo_t = None
                for which, pvt in ((0, pv_e), (1, pv_o)):
                    # row HD of pvt = sum_k exp = softmax denominator (per q).
                    # Copy the unnormalized head out of PSUM immediately so the
                    # pv slot frees for the next head pair; normalize in place
                    # once the reciprocal broadcast lands.
                    rsf = rowp.tile([HD + 1, QC], F32, tag="rsf", name="rsf",
                                    bufs=2)
                    nc.vector.reciprocal(out=rsf[HD:HD + 1, :],
                                         in_=pvt[HD:HD + 1, :])
                    if which == 0:
                        dst = attout[0:HD, hp, :]
                    else:
                        bo_t = tmp.tile([HD, QC], BF, tag="bounce", name="bo_t",
                                        bufs=2)
                        dst = bo_t
                    nc.vector.tensor_copy(out=dst, in_=pvt[0:HD, :])
                    nb = acc.tile([HD, QC], F32, tag="acc", name="nb")
                    nc.tensor.matmul(nb, lhsT=ones_f[HD:HD + 1, 0:HD],
                                     rhs=rsf[HD:HD + 1, :], start=True, stop=True)
                    nb_sb = tmp.tile([HD, QC], F32, tag="nbsb", name="nb_sb",
                                     bufs=2)
                    nc.vector.tensor_copy(out=nb_sb, in_=nb)
                    nc.vector.tensor_mul(out=dst, in0=dst, in1=nb_sb)
                    if which == 1:
                        nc.sync.dma_start(out=attout[64:128, hp, :], in_=bo_t)

            return qt_sb, qpt, attout

        def emit_oln1(qc, qt_sb, qpt, attout):
            # out projection + residual 1 + LN1 stats
            r1f = resp.tile([128, T, QC], F32, tag="rf", name="r1f")
            r1b = resp.tile([128, T, QC], BF, tag="rb", name="r1b")
            acc1 = lnp.tile([1, 2, QC], F32, tag="accrow", name="acc1")
            wo2 = None
            for m in range(T):
                if m % 2 == 0:
                    wo2 = wkq.tile([128, 2, T, 128], BF, tag="wkq", name="wo2")
                    nc.sync.dma_start(
                        out=wo2,
                        in_=wo[m:m + 2].rearrange("a p t q -> p a t q"))
                wom = wo2[:, m % 2]
                po = acc.tile([128, QC], F32, tag="acc", name="po")
                for t in range(T):
                    nc.tensor.matmul(po, lhsT=wom[:, t, :], rhs=attout[:, t, :],
                                     start=(t == 0), stop=(t == T - 1))
                nc.vector.scalar_tensor_tensor(out=r1f[:, m, :], in0=po,
                                               scalar=cols["bo"][:, m:m + 1],
                                               in1=qt_sb[:, m, :],
                                               op0=ALU.add, op1=ALU.add)
                nc.vector.tensor_copy(out=r1b[:, m, :], in_=r1f[:, m, :])
                sq = sqp.tile([128, QC], BF, tag="sq", name="sq")
                nc.vector.tensor_mul(out=sq, in0=r1b[:, m, :], in1=r1b[:, m, :])
                pstat = stp.tile([1, 2, QC], F32, tag="st", name="pstat")
                nc.tensor.matmul(pstat[:, 0, :], lhsT=invd, rhs=r1b[:, m, :],
                                 start=True, stop=True)
                nc.tensor.matmul(pstat[:, 1, :], lhsT=invd, rhs=sq,
                                 start=True, stop=True)
                if m == 0:
                    nc.vector.tensor_copy(out=acc1, in_=pstat)
                else:
                    nc.vector.tensor_add(out=acc1, in0=acc1, in1=pstat)

            mbc, rbc = ln_rows(acc1, f"a{qc}")
            xb = qts.tile([128, T, QC], BF, tag="xb", name="xb")
            for t in range(T):
                t2 = tmp.tile([128, QC], F32, tag="tmpf", name="t2")
                nc.vector.tensor_sub(out=t2, in0=r1f[:, t, :], in1=mbc)
                t3 = tmp.tile([128, QC], F32, tag="tmpf", name="t3")
                nc.vector.tensor_mul(out=t3, in0=t2, in1=rbc)
                nc.scalar.activation(out=xb[:, t, :], in_=t3, func=AF.Identity,
                                     bias=cols["be1"][:, t:t + 1],
                                     scale=cols["g1"][:, t:t + 1])

            return xb

        def emit_ffn(qc, xb):
            # FFN1: hT[dh, q] = relu(W1^T x + b1), feature-major
            hT = big.tile([128, MF, QC], BF, tag="big", name="hT")
            w12 = None
            for m in range(MF):
                if m % 2 == 0:
                    w12 = wkq.tile([128, 2, T, 128], BF, tag="wkq", name="w12")
                    nc.scalar.dma_start(
                        out=w12,
                        in_=w1[m:m + 2].rearrange("a p t q -> p a t q"))
                w1m = w12[:, m % 2]
                ph = acc.tile([128, QC], F32, tag="acc", name="ph")
                for t in range(T):
                    nc.tensor.matmul(ph, lhsT=w1m[:, t, :], rhs=xb[:, t, :],
                                     start=(t == 0), stop=(t == T - 1))
                nc.scalar.activation(out=hT[:, m, :], in_=ph, func=AF.Relu,
                                     bias=b1_sb[:, m:m + 1], scale=1.0)

            # FFN2 + residual 2 + LN2 stats
            r2f = resp.tile([128, T, QC], F32, tag="rf", name="r2f")
            r2b = resp.tile([128, T, QC], BF, tag="rb", name="r2b")
            acc2 = lnp.tile([1, 2, QC], F32, tag="accrow", name="acc2")
            for m in range(T):
                w2m = w8k.tile([128, MF, 128], BF, tag="w8k", name="w2m")
                nc.sync.dma_start(out=w2m, in_=w2[m])
                pf = acc.tile([128, QC], F32, tag="acc", name="pf")
                for t in range(MF):
                    nc.tensor.matmul(pf, lhsT=w2m[:, t, :], rhs=hT[:, t, :],
                                     start=(t == 0), stop=(t == MF - 1))
                t4 = tmp.tile([128, QC], F32, tag="tmpf", name="t4")
                nc.scalar.activation(out=t4, in_=pf, func=AF.Identity,
                                     bias=cols["b2"][:, m:m + 1], scale=1.0)
                nc.vector.tensor_add(out=r2f[:, m, :], in0=t4, in1=xb[:, m, :])
                nc.vector.tensor_copy(out=r2b[:, m, :], in_=r2f[:, m, :])
                sq2 = sqp.tile([128, QC], BF, tag="sq", name="sq2")
                nc.vector.tensor_mul(out=sq2, in0=r2b[:, m, :], in1=r2b[:, m, :])
                pstat2 = stp.tile([1, 2, QC], F32, tag="st", name="pstat2")
                nc.tensor.matmul(pstat2[:, 0, :], lhsT=invd, rhs=r2b[:, m, :],
                                 start=True, stop=True)
                nc.tensor.matmul(pstat2[:, 1, :], lhsT=invd, rhs=sq2,
                                 start=True, stop=True)
                if m == 0:
                    nc.vector.tensor_copy(out=acc2, in_=pstat2)
                else:
                    nc.vector.tensor_add(out=acc2, in0=acc2, in1=pstat2)

            mbc2, rbc2 = ln_rows(acc2, f"b{qc}")
            for t in range(T):
                t2b = tmp.tile([128, QC], F32, tag="tmpf", name="t2b")
                nc.vector.tensor_sub(out=t2b, in0=r2f[:, t, :], in1=mbc2)
                t3b = tmp.tile([128, QC], F32, tag="tmpf", name="t3b")
                nc.vector.tensor_mul(out=t3b, in0=t2b, in1=rbc2)
                ot = outp.tile([128, QC], F32, tag="ot", name="ot")
                nc.scalar.activation(out=ot, in_=t3b, func=AF.Identity,
                                     bias=cols["be2"][:, t:t + 1],
                                     scale=cols["g2"][:, t:t + 1])
                nc.gpsimd.dma_start(out=out_h[t, qc], in_=ot)


        sa = emit_qa(0)
        xb0 = emit_oln1(0, *sa)
        sb = emit_qa(1)
        emit_ffn(0, xb0)
        xb1 = emit_oln1(1, *sb)
        emit_ffn(1, xb1)

    nc.finalize()
    return nc


def get_program():
    if "nc" not in _BUILt_CACHE:
        _BUILt_CACHE["nc"] = _build_program()
    return _BUILt_CACHE["nc"]


def _prep_inputs(q, kv, kv_mask, Wq, bq, Wk, bk, Wv, bv, Wo, bo,
                 ln1_g, ln1_b, W1, b1, W2, b2, ln2_g, ln2_b):
    bf = ml_dtypes.bfloat16
    f32 = np.float32

    def wtiles(W, tq):  # [din, dout] -> [m, p, t, q] with q-width tq
        din, dout = W.shape
        t_, m_ = din // 128, dout // tq
        return np.ascontiguousarray(
            W.reshape(t_, 128, m_, tq).transpose(2, 1, 0, 3)).astype(bf)

    def colf(v):  # [N] -> [128, N//128] (p, t)
        return np.ascontiguousarray(v.reshape(-1, 128).T).astype(f32)

    shared = {
        "wq": wtiles(Wq, 128), "wk": wtiles(Wk, 128), "wv": wtiles(Wv, 512),
        "wo": wtiles(Wo, 128), "w1": wtiles(W1, 128), "w2": wtiles(W2, 128),
        "bq": colf(bq), "bk": colf(bk), "bv": bv.astype(f32), "bo": colf(bo),
        "b1": colf(b1), "b2": colf(b2),
        "g1": colf(ln1_g), "be1": colf(ln1_b),
        "g2": colf(ln2_g), "be2": colf(ln2_b),
    }

    # mask -> additive bias with the reference's all-pad fix
    mask = np.array(kv_mask, dtype=bool).copy()
    all_pad = mask.all(axis=1)
    mask[all_pad, -1] = False
    mbias = np.where(mask, f32(-10000.0), f32(0.0))  # [B, Lk]

    B = q.shape[0]
    in_maps = []
    for b in range(B):
        qT = np.ascontiguousarray(q[b].T)            # [D, L] f32
        kvT = np.ascontiguousarray(kv[b].T)          # [D, L] f32
        m = dict(shared)
        # [qc, p, t, l]
        m["qt_bf"] = np.ascontiguousarray(
            qT.reshape(T, 128, NQC, QC).transpose(2, 1, 0, 3)).astype(bf)
        m["kvt"] = kvT.reshape(T, 128, L).astype(bf)
        m["mb"] = np.ascontiguousarray(mbias[b].reshape(T, 128).T).astype(f32)
        in_maps.append(m)
    return in_maps


def _gather(results):
    outs = []
    for r in results:
        o = r["out"]  # [T, NQC, 128, QC] = outT tiled
        outT = o.transpose(0, 2, 1, 3).reshape(D, L)
        outs.append(outT.T.astype(np.float32))  # back to [L, D]
    return np.stack(outs, axis=0)


def kernel(**inputs):
    from concourse.bass_utils import run_bass_kernel_spmd
    nc = get_program()
    in_maps = _prep_inputs(**inputs)
    res = run_bass_kernel_spmd(nc, in_maps, list(range(NCORES)))
    return _gather(res.results)


def run_traced(**inputs):
    """Like kernel(), but also returns the BassKernelResults with profile."""
    from concourse.bass_utils import run_bass_kernel_spmd
    nc = get_program()
    in_maps = _prep_inputs(**inputs)
    res = run_bass_kernel_spmd(nc, in_maps, list(range(NCORES)), trace=True)
    return _gather(res.results), res

